# revision 22
# baseline (speedup 1.0000x reference)
"""Trainium2 Bass kernel for LoraLinear:
    out = x @ W^T + 2.0 * (x @ A^T) @ B^T
    x: [4, 2048, 4096] f32, W: [4096, 4096], A: [64, 4096], B: [4096, 64]

The LoRA update is folded into the weight on the host (merged-LoRA
inference): out = x @ (W + 2*B@A)^T, exactly. The device runs a pure
[8192 x 4096] @ [4096 x 4096] GEMM.

Sharding across 8 NeuronCores: 4-way data-parallel over tokens x 2-way
tensor-parallel over out-features. Each core computes a [2048 x 2048]
output block. No collectives; the host scatters shards and gathers blocks.

Precision/speed: the fp16 tensor-engine stream rate (1 column/cycle,
measured 216 ns per 512-wide matmul) is the hard floor for a pure fp16
kernel (~448 us/core). The PE's fp8 DoubleRow perf mode (e4m3, 2 weights
per cell, 2 multiplies/cycle) runs the same work in half the instructions,
but pure-e4m3 quantization error (3.75e-2 rel_l2) fails the 2e-2 gate.
The gate is a GLOBAL rel_l2, so a fraction f of the contraction runs in
e4m3 DoubleRow and the rest in fp16: error = base * sqrt(f), with base
measured 3.2e-2 on the real data. With f = 5/16 (k >= 2816 in fp8, as 5
DoubleRow blocks of 256), rel_l2 ~ 1.8e-2 and per-tile matmuls drop
32 -> 27 (15.6% less PE time).

All inputs are pre-scaled by powers of two (x*2, W'*128) - exact in fp16,
near-optimal quantization range for e4m3 - so fp16 and fp8 products
accumulate consistently in PSUM; the DVE copy-out multiplies by 2^-8.

Per-core device program (SPMD, same program on all 8 cores):
  - Merged W'^T resident in SBUF: 22 fp16 k-blocks (88 KB/partition) +
    5 fp8 DoubleRow k-blocks (20 KB/partition).
  - x^T streams in 8 groups of 256 tokens (fp16 part + fp8 part).
  - Per 128-token tile: the 4 o-tiles' fp16 matmuls (22 each, into 4 PSUM
    banks), then all 20 DoubleRow matmuls as one burst (DR LDWEIGHTS only
    pipelines behind other DR matmuls), DVE scaled-copy (x 2^-8) to SBUF,
    store on the SP queue.
  - Startup: ~3.5 us of dummy warmup matmuls issue immediately (PE HAM
    clock-gate warms at 1.2->2.4 GHz after ~3.4 us of activity) while DMAs
    ramp; group 0 then runs k-OUTER across all 8 PSUM banks consuming W'
    k-blocks as they arrive (first blocks split into o-slices so the first
    real matmul starts as early as possible); W DMAs alternate queues by
    k parity; group 1's x prefetch is held behind the W stream.
"""

import numpy as np
import ml_dtypes

import concourse.mybir as mybir
import concourse.tile as tile
from concourse import bacc
from concourse.bass_utils import run_bass_kernel_spmd

# problem dims (hardcoded per harness contract)
B, S, D_IN, D_OUT, R = 4, 2048, 4096, 4096, 64
SCALING = 2.0

T_TOTAL = B * S  # 8192 tokens
DP, TP = 4, 2  # token-parallel x feature-parallel over 8 cores
T_CORE = T_TOTAL // DP  # 2048
O_CORE = D_OUT // TP  # 2048
K = D_IN  # 4096

P = 128  # SBUF partitions
M8 = 5  # fp8 DoubleRow k-blocks for o-tiles 2,3 (256 contraction each)
M8E = 6  # ... and 6 for o-tiles 0,1 (f = 11/32 globally, rel_l2 ~1.87e-2)
KB16 = 32 - 2 * M8  # fp16 k-blocks in x (128 each); x16 covers k < 2816
K16 = KB16 * P  # 2816
K8LO = K - M8E * 256  # 2560: x8/w8 cover k >= 2560 (6 blocks)
TG = 8  # x token groups per core (256 tokens each)
TGW = 256  # tokens per group
NO = 512  # o-tile width (one PSUM bank of fp32)
OT = O_CORE // NO  # 4
N_WARM = 14  # dummy warmup matmuls (issued during DMA ramp)
W_SPLIT = 2  # first fp16 W blocks DMA'd as o-slices for early start

SX = 2.0  # x scale (exact in fp16; good e4m3 range)
SW = 128.0  # W' scale
OUT_SCALE = 1.0 / (SX * SW)  # 2^-8, folded into the DVE copy
E4_CLIP = 448.0  # e4m3 max; values stay well inside

F16 = mybir.dt.float16
F8 = mybir.dt.float8e4
F32 = mybir.dt.float32

_NC_CACHE = {}


def _build_program():
    nc = bacc.Bacc()
    # fp16 x: xq16[g] = [128p, kb*256+t], value x^T[kb*128+p, g*256+t]*SX
    xq16 = nc.declare_dram_parameter("xq16", [TG, P, KB16 * TGW], F16, isOutput=False)
    # fp8 x: xq8[g] = [128p, b*512 + j*256 + t], value x^T[K8LO+b*256+j*128+p, .]*SX
    xq8 = nc.declare_dram_parameter("xq8", [TG, P, M8E * 2 * TGW], F8, isOutput=False)
    # fp16 W: wt16[kb] = [128p, 2048o], value W'^T[kb*128+p, o]*SW
    wt16 = nc.declare_dram_parameter("wt16", [KB16, P, O_CORE], F16, isOutput=False)
    # fp8 W: wt8[b] = [128p, (o, 2j, 512n)], value W'^T[K8LO+b*256+j*128+p, o*512+n]*SW
    wt8 = nc.declare_dram_parameter("wt8", [M8E, P, OT * 2 * NO], F8, isOutput=False)
    out = nc.declare_dram_parameter("out", [T_CORE, O_CORE], F32, isOutput=True)

    DR = mybir.MatmulPerfMode.DoubleRow

    with tile.TileContext(nc) as tc:
        with (
            tc.tile_pool(name="wres", bufs=1) as wres,
            tc.tile_pool(name="xin16", bufs=2) as xin16,
            tc.tile_pool(name="xin8", bufs=2) as xin8,
            tc.tile_pool(name="warm", bufs=1) as warm,
            tc.tile_pool(name="ostage", bufs=4) as ostage,
            tc.tile_pool(name="psacc", bufs=8, space="PSUM") as psacc,
        ):
            wtile16 = wres.tile([P, KB16 * O_CORE], F16, name="wtile16")
            wtile8 = wres.tile([P, M8E * OT, 2, NO], F8, name="wtile8")

            def kb16_range(o):
                # o-tiles 0,1 take a 6th DoubleRow block instead of fp16
                # k-blocks 20,21
                return range(KB16 - 2 if o < 2 else KB16)

            def b8_range(o):
                return range(M8E) if o < 2 else range(1, M8E)
            xt16s, xt8s = {}, {}

            def w16_slice(kb, o):
                return wtile16[:, kb * O_CORE + o * NO : kb * O_CORE + (o + 1) * NO]

            def w8_slice(b, o):
                return wtile8[:, b * OT + o]

            def x16_slice(g, jj, kb):
                return xt16s[g][:, kb * TGW + jj * P : kb * TGW + (jj + 1) * P]

            def x8_slice(g, jj, b):
                return (
                    xt8s[g][:, b * 2 * TGW : (b + 1) * 2 * TGW]
                    .rearrange("p (two t) -> p two t", two=2)[
                        :, :, jj * P : (jj + 1) * P
                    ]
                )

            def load_x(g, after=None):
                xt16_ = xin16.tile([P, KB16 * TGW], F16, name="xt16", tag="xt16")
                xt8_ = xin8.tile([P, M8E * 2 * TGW], F8, name="xt8", tag="xt8")
                dma = nc.scalar.dma_start(out=xt16_[:], in_=xq16[g])
                if after is not None:
                    tile.add_dep_helper(
                        dma.ins, after.ins, reason="x prefetch throttle"
                    )
                nc.scalar.dma_start(out=xt8_[:], in_=xq8[g])
                xt16s[g], xt8s[g] = xt16_, xt8_

            def finish_tile(g, jj, o, ps):
                osb = ostage.tile([P, NO], F32, name="osb")
                nc.vector.tensor_scalar_mul(osb[:], ps[:], OUT_SCALE)
                t = g * 2 + jj
                nc.sync.dma_start(
                    out=out[t * P : (t + 1) * P, o * NO : (o + 1) * NO],
                    in_=osb[:],
                )

            def half_group_pass(g, jj):
                """All 4 o-tiles of one 128-token tile: the fp16 phase per
                o-tile, then all DoubleRow matmuls as ONE burst (the 256-col
                DR LDWEIGHTS only pipelines cleanly behind other DR matmuls;
                interleaving it with the fp16 stream costs ~190ns each)."""
                pss = [psacc.tile([P, NO], F32, name="ps", tag="ps") for _ in range(OT)]
                for o in range(OT):
                    for kb in kb16_range(o):
                        nc.tensor.matmul(
                            pss[o][:], x16_slice(g, jj, kb), w16_slice(kb, o),
                            start=(kb == 0), stop=False,
                        )
                for o in range(OT):
                    for b in b8_range(o):
                        nc.tensor.matmul(
                            pss[o][:], x8_slice(g, jj, b), w8_slice(b, o),
                            start=False, stop=(b == M8E - 1), perf_mode=DR,
                        )
                for o in range(OT):
                    finish_tile(g, jj, o, pss[o])

            # --- PE warmup: dummy matmuls issue immediately (no DMA deps) so
            # the HAM clock-gate is at 2.4 GHz by the time real work starts.
            wmt = warm.tile([P, NO], F16, name="wmt")
            nc.vector.memset(wmt[:], 0.0)
            start_ps = {
                (jj, o): psacc.tile([P, NO], F32, name="ps", tag="ps")
                for jj in range(2)
                for o in range(OT)
            }
            for _ in range(N_WARM):
                nc.tensor.matmul(start_ps[0, 0][:], wmt[:, :P], wmt[:], start=True, stop=True)

            # --- startup DMAs: group-0 x + resident W interleaved on both
            # queues in exact k-outer consumption order; first 3 fp16 W
            # blocks split into o-slices so the first matmul starts early.
            xt16_0 = xin16.tile([P, KB16 * TGW], F16, name="xt16", tag="xt16")
            xt8_0 = xin8.tile([P, M8E * 2 * TGW], F8, name="xt8", tag="xt8")
            w_last_dma = None
            # Startup DMA order = exact consumption order, full-size
            # transfers (small sliced DMAs drop aggregate HBM bandwidth from
            # ~350 to ~260 GB/s - ~600ns queue servicing per descriptor):
            # W16 kb0-1 o-sliced for the earliest possible first matmul,
            # then full 512KB W16 blocks, then the batched W8 stream the DR
            # phase needs last. x-g0 chunks ride along at kb granularity.
            def x16_chunk(eng, kb):
                eng.dma_start(
                    out=xt16_0[:, kb * TGW : (kb + 1) * TGW],
                    in_=xq16[0][:, kb * TGW : (kb + 1) * TGW],
                )

            for kb in range(W_SPLIT):
                eng = nc.sync if kb % 2 == 0 else nc.scalar
                x16_chunk(eng, kb)
                for o in range(OT):
                    eng.dma_start(
                        out=w16_slice(kb, o),
                        in_=wt16[kb][:, o * NO : (o + 1) * NO],
                    )
            for kb in range(W_SPLIT, KB16):
                eng = nc.sync if kb % 2 == 0 else nc.scalar
                x16_chunk(eng, kb)
                if kb < KB16 - 2:
                    eng.dma_start(
                        out=wtile16[:, kb * O_CORE : (kb + 1) * O_CORE], in_=wt16[kb]
                    )
                else:  # kb 20,21: only o-tiles 2,3 read the fp16 block
                    eng.dma_start(
                        out=wtile16[:, kb * O_CORE + 2 * NO : (kb + 1) * O_CORE],
                        in_=wt16[kb][:, 2 * NO :],
                    )
            for b in range(M8E):
                eng = nc.sync if b % 2 == 0 else nc.scalar
                eng.dma_start(
                    out=xt8_0[:, b * 2 * TGW : (b + 1) * 2 * TGW],
                    in_=xq8[0][:, b * 2 * TGW : (b + 1) * 2 * TGW],
                )
                if b == 0:  # only o-tiles 0,1 use block 0
                    w_last_dma = eng.dma_start(
                        out=wtile8[:, 0:2], in_=wt8[0][:, : 2 * 2 * NO]
                    )
                else:
                    w_last_dma = eng.dma_start(
                        out=wtile8[:, b * OT : (b + 1) * OT], in_=wt8[b]
                    )
            xt16s[0], xt8s[0] = xt16_0, xt8_0

            # --- group 0: k-OUTER across all 8 PSUM banks, consuming W'
            # blocks as they arrive instead of waiting for the full weight.
            for kb in range(KB16):
                for o in (range(OT) if kb < KB16 - 2 else range(2, OT)):
                    for jj in range(2):
                        nc.tensor.matmul(
                            start_ps[jj, o][:], x16_slice(0, jj, kb), w16_slice(kb, o),
                            start=(kb == 0), stop=False,
                        )
            # DR phase bank-outer (W8 is resident by now): banks finish
            # staggered, so their DVE copies overlap the next bank's matmuls
            # instead of bursting at the group boundary (which idled the PE
            # ~6us and let the HAM clock-gate re-throttle).
            for jj in range(2):
                for o in range(OT):
                    for b in b8_range(o):
                        nc.tensor.matmul(
                            start_ps[jj, o][:], x8_slice(0, jj, b), w8_slice(b, o),
                            start=False, stop=(b == M8E - 1), perf_mode=DR,
                        )
                    finish_tile(0, jj, o, start_ps[jj, o])

            # --- steady state; group 1's x load is held behind the W stream
            for g in range(1, TG):
                load_x(g, after=w_last_dma if g == 1 else None)
                for jj in range(2):
                    half_group_pass(g, jj)
    return nc


def _get_program():
    if "nc" not in _NC_CACHE:
        nc = _build_program()
        nc.finalize()
        _NC_CACHE["nc"] = nc
    return _NC_CACHE["nc"]


def _prep_x_shard(xs):
    """xs: [T_CORE, K] f32 -> (xq16 [TG*KB16,128,256] f16, xq8 [TG*M8,128,2,256] e4m3)."""
    xs = xs * SX
    x16 = xs[:, :K16].reshape(TG, TGW, KB16, P)  # [g, t, kb, p]
    x16 = np.ascontiguousarray(x16.transpose(0, 3, 2, 1))  # [g, p, kb, t]
    xq16 = x16.astype(np.float16).reshape(TG, P, KB16 * TGW)

    x8 = xs[:, K8LO:].reshape(TG, TGW, M8E, 2, P)  # [g, t, b, j, p]
    x8 = np.ascontiguousarray(x8.transpose(0, 4, 2, 3, 1))  # [g, p, b, j, t]
    xq8 = x8.astype(ml_dtypes.float8_e4m3).reshape(TG, P, M8E * 2 * TGW)
    return xq16, xq8


def _prep_w_shard(wm):
    """wm: [O_CORE, K] f32 merged W' shard -> (wt16 [KB16,128,2048] f16,
    wt8 [M8*OT,128,2,512] e4m3)."""
    wtr = np.ascontiguousarray(wm.T) * SW  # [K, O_CORE]
    wt16 = wtr[:K16].reshape(KB16, P, O_CORE).astype(np.float16)

    w8 = wtr[K8LO:].reshape(M8E, 2, P, OT, NO)  # [b, j, p, o, n]
    w8 = np.ascontiguousarray(w8.transpose(0, 2, 3, 1, 4))  # [b, p, o, j, n]
    wt8 = w8.astype(ml_dtypes.float8_e4m3).reshape(M8E, P, OT * 2 * NO)
    return wt16, wt8


def _prep_in_maps(x, weight, lora_A, lora_B):
    xf = np.ascontiguousarray(x.reshape(T_TOTAL, K))
    w_merged = weight + SCALING * (lora_B @ lora_A)

    x_shards = [_prep_x_shard(xf[d * T_CORE : (d + 1) * T_CORE]) for d in range(DP)]
    w_shards = [
        _prep_w_shard(w_merged[tp * O_CORE : (tp + 1) * O_CORE]) for tp in range(TP)
    ]

    in_maps = []
    for core in range(8):
        d, tp = core // TP, core % TP
        xq16, xq8 = x_shards[d]
        wt16, wt8 = w_shards[tp]
        in_maps.append({"xq16": xq16, "xq8": xq8, "wt16": wt16, "wt8": wt8})
    return in_maps


def _gather(results):
    out = np.empty((T_TOTAL, D_OUT), dtype=np.float32)
    for core in range(8):
        d, tp = core // TP, core % TP
        out[d * T_CORE : (d + 1) * T_CORE, tp * O_CORE : (tp + 1) * O_CORE] = results[
            core
        ]["out"]
    return out.reshape(B, S, D_OUT)


def run(x, weight, lora_A, lora_B, trace=False):
    """Returns (output, BassKernelResults)."""
    nc = _get_program()
    in_maps = _prep_in_maps(
        np.asarray(x, dtype=np.float32),
        np.asarray(weight, dtype=np.float32),
        np.asarray(lora_A, dtype=np.float32),
        np.asarray(lora_B, dtype=np.float32),
    )
    res = run_bass_kernel_spmd(nc, in_maps, list(range(8)), trace=trace)
    return _gather(res.results), res


def kernel(x, weight, lora_A, lora_B):
    out, _ = run(x, weight, lora_A, lora_B, trace=False)
    return out


# revision 23
# speedup vs baseline: 1.0213x; 1.0213x over previous
"""Trainium2 Bass kernel for LoraLinear:
    out = x @ W^T + 2.0 * (x @ A^T) @ B^T
    x: [4, 2048, 4096] f32, W: [4096, 4096], A: [64, 4096], B: [4096, 64]

The LoRA update is folded into the weight on the host (merged-LoRA
inference): out = x @ (W + 2*B@A)^T, exactly. The device runs a pure
[8192 x 4096] @ [4096 x 4096] GEMM.

Sharding across 8 NeuronCores: 4-way data-parallel over tokens x 2-way
tensor-parallel over out-features. Each core computes a [2048 x 2048]
output block. No collectives; the host scatters shards and gathers blocks.

Precision/speed: the fp16 tensor-engine stream rate (1 column/cycle,
measured 216 ns per 512-wide matmul) is the hard floor for a pure fp16
kernel (~448 us/core). The PE's fp8 DoubleRow perf mode (e4m3, 2 weights
per cell, 2 multiplies/cycle) runs the same work in half the instructions,
but pure-e4m3 quantization error (3.75e-2 rel_l2) fails the 2e-2 gate.
The gate is a GLOBAL rel_l2, so a fraction f of the contraction runs in
e4m3 DoubleRow and the rest in fp16: error = base * sqrt(f), with base
measured 3.2e-2 on the real data. With f = 5/16 (k >= 2816 in fp8, as 5
DoubleRow blocks of 256), rel_l2 ~ 1.8e-2 and per-tile matmuls drop
32 -> 27 (15.6% less PE time).

All inputs are pre-scaled by powers of two (x*2, W'*128) - exact in fp16,
near-optimal quantization range for e4m3 - so fp16 and fp8 products
accumulate consistently in PSUM; the DVE copy-out multiplies by 2^-8.

Per-core device program (SPMD, same program on all 8 cores):
  - Merged W'^T resident in SBUF: 22 fp16 k-blocks (88 KB/partition) +
    5 fp8 DoubleRow k-blocks (20 KB/partition).
  - x^T streams in 8 groups of 256 tokens (fp16 part + fp8 part).
  - Per 128-token tile: the 4 o-tiles' fp16 matmuls (22 each, into 4 PSUM
    banks), then all 20 DoubleRow matmuls as one burst (DR LDWEIGHTS only
    pipelines behind other DR matmuls), DVE scaled-copy (x 2^-8) to SBUF,
    store on the SP queue.
  - Startup: ~3.5 us of dummy warmup matmuls issue immediately (PE HAM
    clock-gate warms at 1.2->2.4 GHz after ~3.4 us of activity) while DMAs
    ramp; group 0 then runs k-OUTER across all 8 PSUM banks consuming W'
    k-blocks as they arrive (first blocks split into o-slices so the first
    real matmul starts as early as possible); W DMAs alternate queues by
    k parity; group 1's x prefetch is held behind the W stream.
"""

import numpy as np
import ml_dtypes

import concourse.mybir as mybir
import concourse.tile as tile
from concourse import bacc
from concourse.bass_utils import run_bass_kernel_spmd

# problem dims (hardcoded per harness contract)
B, S, D_IN, D_OUT, R = 4, 2048, 4096, 4096, 64
SCALING = 2.0

T_TOTAL = B * S  # 8192 tokens
DP, TP = 4, 2  # token-parallel x feature-parallel over 8 cores
T_CORE = T_TOTAL // DP  # 2048
O_CORE = D_OUT // TP  # 2048
K = D_IN  # 4096

P = 128  # SBUF partitions
M8 = 5  # fp8 DoubleRow k-blocks for o-tiles 2,3 (256 contraction each)
M8E = 6  # ... and 6 for o-tiles 0,1 (f = 11/32 globally, rel_l2 ~1.87e-2)
KB16 = 32 - 2 * M8  # fp16 k-blocks in x (128 each); x16 covers k < 2816
K16 = KB16 * P  # 2816
K8LO = K - M8E * 256  # 2560: x8/w8 cover k >= 2560 (6 blocks)
TG = 8  # x token groups per core (256 tokens each)
TGW = 256  # tokens per group
NO = 512  # o-tile width (one PSUM bank of fp32)
OT = O_CORE // NO  # 4
N_WARM = 8  # dummy warmup matmuls (issued during DMA ramp)
W_SPLIT = 2  # first fp16 W blocks DMA'd as o-slices for early start

SX = 2.0  # x scale (exact in fp16; good e4m3 range)
SW = 128.0  # W' scale
OUT_SCALE = 1.0 / (SX * SW)  # 2^-8, folded into the DVE copy
E4_CLIP = 448.0  # e4m3 max; values stay well inside

F16 = mybir.dt.float16
F8 = mybir.dt.float8e4
F32 = mybir.dt.float32

_NC_CACHE = {}


def _build_program():
    nc = bacc.Bacc()
    # fp16 x: xq16[g] = [128p, kb*256+t], value x^T[kb*128+p, g*256+t]*SX
    xq16 = nc.declare_dram_parameter("xq16", [TG, P, KB16 * TGW], F16, isOutput=False)
    # fp8 x: xq8[g] = [128p, b*512 + j*256 + t], value x^T[K8LO+b*256+j*128+p, .]*SX
    xq8 = nc.declare_dram_parameter("xq8", [TG, P, M8E * 2 * TGW], F8, isOutput=False)
    # fp16 W: wt16[kb] = [128p, 2048o], value W'^T[kb*128+p, o]*SW
    wt16 = nc.declare_dram_parameter("wt16", [KB16, P, O_CORE], F16, isOutput=False)
    # fp8 W: wt8[b] = [128p, (o, 2j, 512n)], value W'^T[K8LO+b*256+j*128+p, o*512+n]*SW
    wt8 = nc.declare_dram_parameter("wt8", [M8E, P, OT * 2 * NO], F8, isOutput=False)
    out = nc.declare_dram_parameter("out", [T_CORE, O_CORE], F32, isOutput=True)

    DR = mybir.MatmulPerfMode.DoubleRow

    with tile.TileContext(nc) as tc:
        with (
            tc.tile_pool(name="wres", bufs=1) as wres,
            tc.tile_pool(name="xin16", bufs=2) as xin16,
            tc.tile_pool(name="xin8", bufs=2) as xin8,
            tc.tile_pool(name="warm", bufs=1) as warm,
            tc.tile_pool(name="ostage", bufs=4) as ostage,
            tc.tile_pool(name="psacc", bufs=8, space="PSUM") as psacc,
        ):
            wtile16 = wres.tile([P, KB16 * O_CORE], F16, name="wtile16")
            wtile8 = wres.tile([P, M8E * OT, 2, NO], F8, name="wtile8")

            def kb16_range(o):
                # o-tiles 0,1 take a 6th DoubleRow block instead of fp16
                # k-blocks 20,21
                return range(KB16 - 2 if o < 2 else KB16)

            def b8_range(o):
                return range(M8E) if o < 2 else range(1, M8E)
            xt16s, xt8s = {}, {}

            def w16_slice(kb, o):
                return wtile16[:, kb * O_CORE + o * NO : kb * O_CORE + (o + 1) * NO]

            def w8_slice(b, o):
                return wtile8[:, b * OT + o]

            def x16_slice(g, jj, kb):
                return xt16s[g][:, kb * TGW + jj * P : kb * TGW + (jj + 1) * P]

            def x8_slice(g, jj, b):
                return (
                    xt8s[g][:, b * 2 * TGW : (b + 1) * 2 * TGW]
                    .rearrange("p (two t) -> p two t", two=2)[
                        :, :, jj * P : (jj + 1) * P
                    ]
                )

            def load_x(g, after=None):
                xt16_ = xin16.tile([P, KB16 * TGW], F16, name="xt16", tag="xt16")
                xt8_ = xin8.tile([P, M8E * 2 * TGW], F8, name="xt8", tag="xt8")
                dma = nc.scalar.dma_start(out=xt16_[:], in_=xq16[g])
                if after is not None:
                    tile.add_dep_helper(
                        dma.ins, after.ins, reason="x prefetch throttle"
                    )
                nc.scalar.dma_start(out=xt8_[:], in_=xq8[g])
                xt16s[g], xt8s[g] = xt16_, xt8_

            def finish_tile(g, jj, o, ps):
                osb = ostage.tile([P, NO], F32, name="osb")
                nc.vector.tensor_scalar_mul(osb[:], ps[:], OUT_SCALE)
                t = g * 2 + jj
                nc.sync.dma_start(
                    out=out[t * P : (t + 1) * P, o * NO : (o + 1) * NO],
                    in_=osb[:],
                )

            def half_group_pass(g, jj):
                """All 4 o-tiles of one 128-token tile: the fp16 phase per
                o-tile, then all DoubleRow matmuls as ONE burst (the 256-col
                DR LDWEIGHTS only pipelines cleanly behind other DR matmuls;
                interleaving it with the fp16 stream costs ~190ns each)."""
                pss = [psacc.tile([P, NO], F32, name="ps", tag="ps") for _ in range(OT)]
                for o in range(OT):
                    for kb in kb16_range(o):
                        nc.tensor.matmul(
                            pss[o][:], x16_slice(g, jj, kb), w16_slice(kb, o),
                            start=(kb == 0), stop=False,
                        )
                for o in range(OT):
                    for b in b8_range(o):
                        nc.tensor.matmul(
                            pss[o][:], x8_slice(g, jj, b), w8_slice(b, o),
                            start=False, stop=(b == M8E - 1), perf_mode=DR,
                        )
                for o in range(OT):
                    finish_tile(g, jj, o, pss[o])

            # --- PE warmup: dummy matmuls issue immediately (no DMA deps) so
            # the HAM clock-gate is at 2.4 GHz by the time real work starts.
            wmt = warm.tile([P, NO], F16, name="wmt")
            nc.vector.memset(wmt[:], 0.0)
            start_ps = {
                (jj, o): psacc.tile([P, NO], F32, name="ps", tag="ps")
                for jj in range(2)
                for o in range(OT)
            }
            for _ in range(N_WARM):
                nc.tensor.matmul(start_ps[0, 0][:], wmt[:, :P], wmt[:], start=True, stop=True)

            # --- startup DMAs: group-0 x + resident W interleaved on both
            # queues in exact k-outer consumption order; first 3 fp16 W
            # blocks split into o-slices so the first matmul starts early.
            xt16_0 = xin16.tile([P, KB16 * TGW], F16, name="xt16", tag="xt16")
            xt8_0 = xin8.tile([P, M8E * 2 * TGW], F8, name="xt8", tag="xt8")
            w_last_dma = None
            # Startup DMA order = exact consumption order, full-size
            # transfers (small sliced DMAs drop aggregate HBM bandwidth from
            # ~350 to ~260 GB/s - ~600ns queue servicing per descriptor):
            # W16 kb0-1 o-sliced for the earliest possible first matmul,
            # then full 512KB W16 blocks, then the batched W8 stream the DR
            # phase needs last. x-g0 chunks ride along at kb granularity.
            def x16_chunk(eng, kb):
                eng.dma_start(
                    out=xt16_0[:, kb * TGW : (kb + 1) * TGW],
                    in_=xq16[0][:, kb * TGW : (kb + 1) * TGW],
                )

            for kb in range(W_SPLIT):
                eng = nc.sync if kb % 2 == 0 else nc.scalar
                x16_chunk(eng, kb)
                for o in range(OT):
                    eng.dma_start(
                        out=w16_slice(kb, o),
                        in_=wt16[kb][:, o * NO : (o + 1) * NO],
                    )
            for kb in range(W_SPLIT, KB16):
                eng = nc.sync if kb % 2 == 0 else nc.scalar
                x16_chunk(eng, kb)
                if kb < KB16 - 2:
                    eng.dma_start(
                        out=wtile16[:, kb * O_CORE : (kb + 1) * O_CORE], in_=wt16[kb]
                    )
                else:  # kb 20,21: only o-tiles 2,3 read the fp16 block
                    eng.dma_start(
                        out=wtile16[:, kb * O_CORE + 2 * NO : (kb + 1) * O_CORE],
                        in_=wt16[kb][:, 2 * NO :],
                    )
            for b in range(M8E):
                eng = nc.sync if b % 2 == 0 else nc.scalar
                eng.dma_start(
                    out=xt8_0[:, b * 2 * TGW : (b + 1) * 2 * TGW],
                    in_=xq8[0][:, b * 2 * TGW : (b + 1) * 2 * TGW],
                )
                if b == 0:  # only o-tiles 0,1 use block 0
                    w_last_dma = eng.dma_start(
                        out=wtile8[:, 0:2], in_=wt8[0][:, : 2 * 2 * NO]
                    )
                else:
                    w_last_dma = eng.dma_start(
                        out=wtile8[:, b * OT : (b + 1) * OT], in_=wt8[b]
                    )
            xt16s[0], xt8s[0] = xt16_0, xt8_0

            # --- group 0: k-OUTER across all 8 PSUM banks, consuming W'
            # blocks as they arrive instead of waiting for the full weight.
            for kb in range(KB16):
                for o in (range(OT) if kb < KB16 - 2 else range(2, OT)):
                    for jj in range(2):
                        nc.tensor.matmul(
                            start_ps[jj, o][:], x16_slice(0, jj, kb), w16_slice(kb, o),
                            start=(kb == 0), stop=False,
                        )
            # DR phase bank-outer (W8 is resident by now): banks finish
            # staggered, so their DVE copies overlap the next bank's matmuls
            # instead of bursting at the group boundary (which idled the PE
            # ~6us and let the HAM clock-gate re-throttle).
            for jj in range(2):
                for o in range(OT):
                    for b in b8_range(o):
                        nc.tensor.matmul(
                            start_ps[jj, o][:], x8_slice(0, jj, b), w8_slice(b, o),
                            start=False, stop=(b == M8E - 1), perf_mode=DR,
                        )
                    finish_tile(0, jj, o, start_ps[jj, o])

            # --- steady state; group 1's x load is held behind the W stream
            for g in range(1, TG):
                load_x(g, after=w_last_dma if g == 1 else None)
                for jj in range(2):
                    half_group_pass(g, jj)
    return nc


def _get_program():
    if "nc" not in _NC_CACHE:
        nc = _build_program()
        nc.finalize()
        _NC_CACHE["nc"] = nc
    return _NC_CACHE["nc"]


def _prep_x_shard(xs):
    """xs: [T_CORE, K] f32 -> (xq16 [TG*KB16,128,256] f16, xq8 [TG*M8,128,2,256] e4m3)."""
    xs = xs * SX
    x16 = xs[:, :K16].reshape(TG, TGW, KB16, P)  # [g, t, kb, p]
    x16 = np.ascontiguousarray(x16.transpose(0, 3, 2, 1))  # [g, p, kb, t]
    xq16 = x16.astype(np.float16).reshape(TG, P, KB16 * TGW)

    x8 = xs[:, K8LO:].reshape(TG, TGW, M8E, 2, P)  # [g, t, b, j, p]
    x8 = np.ascontiguousarray(x8.transpose(0, 4, 2, 3, 1))  # [g, p, b, j, t]
    xq8 = x8.astype(ml_dtypes.float8_e4m3).reshape(TG, P, M8E * 2 * TGW)
    return xq16, xq8


def _prep_w_shard(wm):
    """wm: [O_CORE, K] f32 merged W' shard -> (wt16 [KB16,128,2048] f16,
    wt8 [M8*OT,128,2,512] e4m3)."""
    wtr = np.ascontiguousarray(wm.T) * SW  # [K, O_CORE]
    wt16 = wtr[:K16].reshape(KB16, P, O_CORE).astype(np.float16)

    w8 = wtr[K8LO:].reshape(M8E, 2, P, OT, NO)  # [b, j, p, o, n]
    w8 = np.ascontiguousarray(w8.transpose(0, 2, 3, 1, 4))  # [b, p, o, j, n]
    wt8 = w8.astype(ml_dtypes.float8_e4m3).reshape(M8E, P, OT * 2 * NO)
    return wt16, wt8


def _prep_in_maps(x, weight, lora_A, lora_B):
    xf = np.ascontiguousarray(x.reshape(T_TOTAL, K))
    w_merged = weight + SCALING * (lora_B @ lora_A)

    x_shards = [_prep_x_shard(xf[d * T_CORE : (d + 1) * T_CORE]) for d in range(DP)]
    w_shards = [
        _prep_w_shard(w_merged[tp * O_CORE : (tp + 1) * O_CORE]) for tp in range(TP)
    ]

    in_maps = []
    for core in range(8):
        d, tp = core // TP, core % TP
        xq16, xq8 = x_shards[d]
        wt16, wt8 = w_shards[tp]
        in_maps.append({"xq16": xq16, "xq8": xq8, "wt16": wt16, "wt8": wt8})
    return in_maps


def _gather(results):
    out = np.empty((T_TOTAL, D_OUT), dtype=np.float32)
    for core in range(8):
        d, tp = core // TP, core % TP
        out[d * T_CORE : (d + 1) * T_CORE, tp * O_CORE : (tp + 1) * O_CORE] = results[
            core
        ]["out"]
    return out.reshape(B, S, D_OUT)


def run(x, weight, lora_A, lora_B, trace=False):
    """Returns (output, BassKernelResults)."""
    nc = _get_program()
    in_maps = _prep_in_maps(
        np.asarray(x, dtype=np.float32),
        np.asarray(weight, dtype=np.float32),
        np.asarray(lora_A, dtype=np.float32),
        np.asarray(lora_B, dtype=np.float32),
    )
    res = run_bass_kernel_spmd(nc, in_maps, list(range(8)), trace=trace)
    return _gather(res.results), res


def kernel(x, weight, lora_A, lora_B):
    out, _ = run(x, weight, lora_A, lora_B, trace=False)
    return out
